# revision 21
# baseline (speedup 1.0000x reference)
# Trainium2 Bass kernel for nn_Decoder_14568529068506 (gnn_message_passing).
#
# Reference computation (per scene s of 32, P=48 peds):
#   rel[i,j]  = obs[j] - obs[i]                  (P,P,2T)   2T=16
#   emb       = rel @ W_se.T                     (P,P,512)
#   emb      *= tile(traj_weight[s])             (P,P,512)
#   x         = concat([emb, h[j]], -1)          (P,P,576)
#   x1        = relu(x @ W1.T + b1)              (P,P,512)
#   x2        = relu(x1 @ W2.T + b2)             (P,P,1024)
#   out[s,i]  = max_j x2[i,j]                    (P,1024)
#
# Kernel restructuring (validated exactly in fp32 numpy):
#  * The tiled traj_weight multiply + spatial embedding + W1 are fused:
#      out1[d,row] = sum_{(ct,g)} Wf[d,(ct,g)] * tw[row,ct] * rel[row,g]
#    with Wf[d, ct*16+g] = sum_{k%2==c} W1[d, t*64+k] * W_se[t*64+k, g].
#    So MLP1 contracts over 256 "rel2" features (+64 h features) instead
#    of 576, and the (P,P,512) embedding is never materialized.
#  * rel2 = tw_rep * rel_rep is built feature-major on 128 partitions:
#      rel_rep = obs_rep.T @ D   (D = +-1 pairwise difference matrix)
#      tw_rep  = Rb.T @ twT      (Rb = 0/1 replication matrix)
#    i.e. three cheap matmuls + two vector multiplies per row block.
#  * The h-state part of MLP1 rides as a third K-accumulation matmul
#    whose rhs (h broadcast over i) is uploaded once per scene.
#  * relu/bias commute with max-pool, so MLP2 outputs are max-pooled
#    straight out of PSUM; bias+relu are applied post-pool on [128,48].
#  * Matmuls run in bf16 (1 cycle/row; separate LDWEIGHTS path). PSUM
#    accumulation stays fp32; only matmul operands are rounded.
#  * All matmuls use K=128 (operands zero-padded on the host side for
#    the stationary/constant factor; the streamed factor's extra rows
#    are garbage multiplied by zero).  K<128 matmuls with tile_position
#    take a slow SW-decode path on the PE sequencer (~+100ns each).
#
# Sharding: scenes are data-parallel across the 8 cores (4 scenes each);
# weights replicated; the (192,1024) per-core outputs are concatenated on
# the host (no collectives needed).

import numpy as np

S, P, T, E, H = 32, 48, 8, 64, 64
D1, D2 = 512, 1024
B = S * P
NCORES = 8
SC = S // NCORES          # scenes per core
NB = 6                    # row blocks per scene
NBLK = P * P // NB        # 384 columns (pairs) per block = 8 i-groups x 48 j
IB = NBLK // P            # i-groups per block (8)


def _host_constants(W_se, W1, W2, b1, b2):
    """Precompute fused weights + structural constant matrices (fp32)."""
    W_se = np.asarray(W_se, np.float32)
    W1 = np.asarray(W1, np.float32)
    W2 = np.asarray(W2, np.float32)
    b1 = np.asarray(b1, np.float32)
    b2 = np.asarray(b2, np.float32)

    W1e, W1h = W1[:, :512], W1[:, 512:]
    Wf = np.zeros((D1, 256), np.float32)
    for c in range(2):
        for t in range(T):
            ct = c * 8 + t
            f = t * 64 + np.arange(c, 64, 2)
            Wf[:, ct * 16:(ct + 1) * 16] = W1e[:, f] @ W_se[f, :]

    # pairwise difference matrix, zero-padded to K=128 rows
    Dm = np.zeros((128, P * P), np.float32)
    ii, jj = np.meshgrid(np.arange(P), np.arange(P), indexing="ij")
    rows = (ii * P + jj).ravel()
    np.add.at(Dm, (jj.ravel(), rows), 1.0)
    np.add.at(Dm, (ii.ravel(), rows), -1.0)

    # lhsT tile layouts: [128, kTiles, M] so DMAs are contiguous
    Wf_sb = np.ascontiguousarray(Wf.T.reshape(2, 128, D1).transpose(1, 0, 2))
    W1h_sb = np.zeros((128, D1), np.float32)
    W1h_sb[:H] = W1h.T                                       # K=128 zero-pad
    W2_sb = np.ascontiguousarray(W2.T.reshape(4, 128, D2).transpose(1, 0, 2))
    b1_sb = np.ascontiguousarray(b1.reshape(4, 128).T)       # (128, 4)
    b2_sb = np.ascontiguousarray(b2.reshape(8, 128).T)       # (128, 8)
    ident = np.eye(128, dtype=np.float32)
    return dict(Wf_sb=Wf_sb, W1h_sb=W1h_sb, W2_sb=W2_sb, b1_sb=b1_sb,
                b2_sb=b2_sb, Dm=Dm, ident=ident)


def build_program(n_scenes=SC):
    """Emit the per-core Bass/Tile program. Returns the compiled Bacc.

    Built on bacc.Bacc (not raw bass.Bass): Bacc.compile() runs the
    TRN2 sync legalization (move_matmul_waits_to_ldweights +
    generate_event_semaphores) that splits multi-semaphore waits —
    hardware allows at most one sync-wait per instruction.
    """
    from contextlib import ExitStack
    import concourse.bacc as bacc
    import concourse.tile as tile
    from concourse import mybir
    from concourse.alu_op_type import AluOpType

    f32 = mybir.dt.float32
    bf16 = mybir.dt.bfloat16
    AF = mybir.ActivationFunctionType
    AX = mybir.AxisListType

    nc = bacc.Bacc("TRN2", target_bir_lowering=False, debug=False)

    # ---- DRAM parameters -------------------------------------------------
    # bf16 consts blob: Dm(2304) | W1h(512) = 2816 cols
    d_cb = nc.dram_tensor("constsB", [128, 2816], bf16, kind="ExternalInput")
    # f32 consts blob: b1(4) | b2(8) | ident(128) = 140 cols
    d_cf = nc.dram_tensor("constsF", [128, 140], f32, kind="ExternalInput")
    # scene blob: tw_rep0(2304) | tw_rep1(2304) | hj(384) = 4992 cols
    d_blob = nc.dram_tensor("blob", [n_scenes, 128, 4992], bf16, kind="ExternalInput")
    d_obs = nc.dram_tensor("obs_rep", [n_scenes, 128, 128], bf16, kind="ExternalInput")
    d_Wf = nc.dram_tensor("Wf_sb", [128, 2, D1], bf16, kind="ExternalInput")
    d_W2 = nc.dram_tensor("W2_sb", [128, 4, D2], bf16, kind="ExternalInput")
    d_out = nc.dram_tensor("out", [n_scenes * P, D2], f32, kind="ExternalOutput")

    with ExitStack() as ctx:
        tc = ctx.enter_context(tile.TileContext(nc))
        consts = ctx.enter_context(tc.tile_pool(name="consts", bufs=1))
        tw_pool = ctx.enter_context(tc.tile_pool(name="tw", bufs=2))
        scene_pool = ctx.enter_context(tc.tile_pool(name="scene", bufs=2))
        blk_pool = ctx.enter_context(tc.tile_pool(name="blk", bufs=3))
        pp = ctx.enter_context(tc.tile_pool(name="pp", bufs=2, space="PSUM"))
        p1 = ctx.enter_context(tc.tile_pool(name="p1", bufs=2, space="PSUM"))
        p2 = ctx.enter_context(tc.tile_pool(name="p2", bufs=4, space="PSUM"))

        cb = consts.tile([128, 2816], bf16)
        cf = consts.tile([128, 140], f32)
        Dm_sb = cb[:, 0:P * P]
        W1h_sb = cb[:, P * P:P * P + D1]
        b1_sb = cf[:, 0:4]
        b2_sb = cf[:, 4:12]
        id_sb = cf[:, 12:140]
        zero_sb = consts.tile([128, P], f32)
        Wf_sb = consts.tile([128, 2, D1], bf16)
        W2_sb = consts.tile([128, 4, D2], bf16)

        state = {}   # per-scene tiles
        mlp_q = []   # software pipeline: deferred MLP stage

        def scene_setup(s, chunked=False):
            # tiles are [128, ...]: rows past the data are host-zero-padded
            # (hardware SBUF garbage can contain NaN, and 0*NaN = NaN kills
            # the K=128 zero-padding trick otherwise).  obs rides its own
            # small DMA so the first rel matmul isn't gated on the blob.
            obs_rep = scene_pool.tile([128, 128], bf16, tag="obs_rep")
            nc.sync.dma_start(obs_rep[:], d_obs[s])
            blob = tw_pool.tile([128, 4992], bf16, tag="blob")
            if chunked:
                # scene 0 startup: block-0's tw_rep columns + hj first so
                # the first TT/mlp1 aren't gated on the 3.9us full blob
                for lo, hi in ((0, NBLK), (P * P, P * P + NBLK),
                               (2 * P * P, 4992), (NBLK, P * P),
                               (P * P + NBLK, 2 * P * P)):
                    nc.sync.dma_start(blob[:, lo:hi], d_blob[s, :, lo:hi])
            else:
                nc.sync.dma_start(blob[:], d_blob[s])
            pooled = scene_pool.tile([128, 4, 2 * P], f32, tag="pooled")
            state[s] = dict(blob=blob, obs_rep=obs_rep, pooled=pooled)

        def load_weights():
            # weight DMAs dispatch from the (startup-idle) Activation
            # engine queue so they don't serialize behind the scene-0
            # data DMAs on the sync queue
            nc.scalar.dma_start(Wf_sb[:, 0], d_Wf[:, 0])
            nc.scalar.dma_start(Wf_sb[:, 1], d_Wf[:, 1])
            for k in range(4):
                nc.scalar.dma_start(W2_sb[:, k], d_W2[:, k])

        def prep(s, b):
            st = state[s]
            c0 = b * NBLK
            rel_ps = pp.tile([128, NBLK], f32, tag="pp")
            nc.tensor.matmul(rel_ps[:], st["obs_rep"][:],
                             Dm_sb[:, c0:c0 + NBLK], start=True, stop=True)
            # rel eviction on the scalar engine (to bf16) frees DVE cycles;
            # tw_rep comes pre-replicated from DRAM, so both rel2 multiplies
            # run on all-SBUF 2-byte operands (fast DVE mode)
            rel_sb = blk_pool.tile([128, NBLK], bf16, tag="rel_sb")
            nc.scalar.copy(rel_sb[:], rel_ps[:])
            rel2_0 = blk_pool.tile([128, NBLK], bf16, tag="rel2_0")
            nc.vector.tensor_tensor(rel2_0[:], st["blob"][:, c0:c0 + NBLK],
                                    rel_sb[:], AluOpType.mult)
            rel2_1 = blk_pool.tile([128, NBLK], bf16, tag="rel2_1")
            nc.vector.tensor_tensor(rel2_1[:], st["blob"][:, P * P + c0:P * P + c0 + NBLK],
                                    rel_sb[:], AluOpType.mult)
            return dict(rel2_0=rel2_0, rel2_1=rel2_1, s=s, b=b)

        def mlp1(job):
            s, b = job["s"], job["b"]
            st = state[s]
            r20 = job["rel2_0"][:]
            r21 = job["rel2_1"][:]
            x1 = blk_pool.tile([128, 4, NBLK], bf16, tag="x1")
            for m in range(4):
                p1t = p1.tile([128, NBLK], f32, tag="p1")
                nc.tensor.matmul(p1t[:], Wf_sb[:, 0, m * 128:(m + 1) * 128],
                                 r20, start=True, stop=False)
                nc.tensor.matmul(p1t[:], Wf_sb[:, 1, m * 128:(m + 1) * 128],
                                 r21, start=False, stop=False)
                nc.tensor.matmul(p1t[:], W1h_sb[:, m * 128:(m + 1) * 128],
                                 st["blob"][:, 2 * P * P:2 * P * P + NBLK],
                                 start=False, stop=True)
                nc.scalar.activation(x1[:, m, :], p1t[:], AF.Relu,
                                     bias=b1_sb[:, m:m + 1])
            job["x1"] = x1

        def mlp2(job):
            s, b = job["s"], job["b"]
            st = state[s]
            x1 = job["x1"]
            last = b == NB - 1
            for mm in range(8):
                p2t = p2.tile([128, NBLK], f32, tag="p2")
                for k in range(4):
                    nc.tensor.matmul(
                        p2t[:], W2_sb[:, k, mm * 128:(mm + 1) * 128],
                        x1[:, k, :], start=(k == 0), stop=(k == 3))
                dst = st["pooled"][:, mm // 2,
                                   (mm % 2) * P + b * IB:(mm % 2) * P + (b + 1) * IB]
                nc.vector.tensor_reduce(
                    dst, p2t[:].rearrange("p (i j) -> p i j", i=IB),
                    axis=AX.X, op=AluOpType.max)
                # one-group delay so PE doesn't stall on the pair's
                # reduce -> transpose chain (no delay on the final scene,
                # where it would only stretch the kernel tail)
                if last and mm % 2 == 1:
                    if s == n_scenes - 1:
                        finish_pair(s, st, mm // 2)
                    elif mm >= 3:
                        finish_pair(s, st, (mm - 3) // 2)
            if last:
                if s != n_scenes - 1:
                    finish_pair(s, st, 2)
                    finish_pair(s, st, 3)
                state.pop(s)

        def finish_pair(s, st, pi):
            """Scene output for m-tile pair pi: bias+relu post-pool,
            transpose to row-major, DMA straight out of PSUM."""
            pooled = st["pooled"]
            for half in range(2):
                mm = 2 * pi + half
                sl = pooled[:, pi, half * P:(half + 1) * P]
                nc.vector.scalar_tensor_tensor(
                    sl, sl, b2_sb[:, mm:mm + 1], zero_sb[:],
                    op0=AluOpType.add, op1=AluOpType.max)
            tps = p1.tile([128, NBLK], f32, tag="p1")
            nc.tensor.transpose(tps[:2 * P, :128], pooled[:, pi, :], id_sb)
            ot = scene_pool.tile([2 * P, 128], f32, tag="ot")
            nc.vector.tensor_copy(ot[:], tps[:2 * P, :128])
            nc.sync.dma_start(
                d_out[s * P:(s + 1) * P, (2 * pi) * 128:(2 * pi + 1) * 128],
                ot[:P, :])
            nc.sync.dma_start(
                d_out[s * P:(s + 1) * P, (2 * pi + 1) * 128:(2 * pi + 2) * 128],
                ot[P:2 * P, :])

        # startup order: the consts blob holds Dm/Rb needed by the very
        # first prep matmul, so it dispatches first on the sync queue;
        # scene-0 data follows; weights ride the (startup-idle) scalar
        # HWDGE ring in parallel; the STT zero operand memset is on vector
        nc.sync.dma_start(cb[:], d_cb[:])
        scene_setup(0, chunked=True)
        nc.sync.dma_start(cf[:], d_cf[:])
        nc.vector.memset(zero_sb[:], 0.0)
        load_weights()

        # two-deep software pipeline on PE:
        #   ... prep(i)  mlp1(i-1)  mlp2(i-2) ...
        # so x1 is ready a full block before MLP2 consumes it and PSUM
        # slot recycling has a block of slack; scene data is prefetched
        # one block before the scene starts
        blocks = [(s, b) for s in range(n_scenes) for b in range(NB)]
        for idx, (s, b) in enumerate(blocks):
            if b == NB - 2 and s + 1 < n_scenes:
                scene_setup(s + 1)
            mlp_q.append(prep(s, b))
            if len(mlp_q) > 1:
                mlp1(mlp_q[-2])
            if len(mlp_q) > 2:
                mlp2(mlp_q.pop(0))
        mlp1(mlp_q[-1])
        mlp2(mlp_q.pop(0))
        mlp2(mlp_q.pop(0))

    nc.compile()
    return nc


def _host_inputs(h_states, traj, traj_weight, consts, n_scenes=SC):
    """Slice + lay out per-core input maps (matmul operands cast to bf16)."""
    import ml_dtypes
    bf = ml_dtypes.bfloat16
    h_states = np.asarray(h_states, np.float32)
    traj = np.asarray(traj, np.float32)
    traj_weight = np.asarray(traj_weight, np.float32)

    obs_full = np.ascontiguousarray(
        traj[:T].transpose(1, 0, 2).reshape(B, 2 * T))          # (B,16) g=t*2+c
    h_full = h_states.reshape(S, P, H)

    constsB = np.concatenate(
        [consts["Dm"], consts["W1h_sb"]], axis=1).astype(bf)    # (128, 2816)
    constsF = np.concatenate(
        [consts["b1_sb"], consts["b2_sb"], consts["ident"]],
        axis=1).astype(np.float32)                              # (128, 140)
    Wf_sb = consts["Wf_sb"].astype(bf)
    W2_sb = consts["W2_sb"].astype(bf)

    in_maps = []
    for core in range(NCORES):
        s0 = core * n_scenes
        sl = slice(s0, s0 + n_scenes)
        twT = traj_weight[sl].transpose(0, 2, 3, 1).reshape(n_scenes, 16, P * P)
        # blob: tw_rep0 | tw_rep1 | hj.  tw_rep{t}[p] = tw[8t + p//16]
        blob = np.zeros((n_scenes, 128, 4992), np.float32)
        blob[:, :, 0:P * P] = np.repeat(twT[:, 0:8], 16, axis=1)
        blob[:, :, P * P:2 * P * P] = np.repeat(twT[:, 8:16], 16, axis=1)
        # hj[s, d, r*48+j] = h[s, j, d] broadcast over i-groups (rows 64+ zero)
        blob[:, :H, 2 * P * P:] = np.tile(
            h_full[sl].transpose(0, 2, 1).reshape(n_scenes, H, 1, P),
            (1, 1, IB, 1)).reshape(n_scenes, H, NBLK)
        blob = blob.astype(bf)
        # obs_rep[s, j, r*16+g] = obs[j, g] replicated 8x along features
        obs_rep = np.zeros((n_scenes, 128, 128), np.float32)
        obs_rep[:, :P] = np.tile(
            obs_full[s0 * P:(s0 + n_scenes) * P].reshape(n_scenes, P, 1, 2 * T),
            (1, 1, 8, 1)).reshape(n_scenes, P, 128)
        obs_rep = obs_rep.astype(bf)
        m = dict(blob=blob, obs_rep=obs_rep,
                 constsB=constsB, constsF=constsF, Wf_sb=Wf_sb, W2_sb=W2_sb)
        in_maps.append(m)
    return in_maps


def kernel(h_states, seq_start_end, end_pos, traj, traj_weight,
           mlp_pre_pool_dim_0, W_se, b_se, W1, b1, W2, b2):
    import sys
    if '/opt/trn_rl_repo' not in sys.path:
        sys.path.insert(0, '/opt/trn_rl_repo')
    from concourse.bass_utils import run_bass_kernel_spmd

    consts = _host_constants(W_se, W1, W2, b1, b2)
    in_maps = _host_inputs(h_states, traj, traj_weight, consts)
    nc = build_program(SC)
    res = run_bass_kernel_spmd(nc, in_maps, list(range(NCORES)))
    out = np.concatenate([res.results[i]["out"] for i in range(NCORES)], axis=0)
    return out.astype(np.float32)


# revision 26
# speedup vs baseline: 1.0262x; 1.0262x over previous
# Trainium2 Bass kernel for nn_Decoder_14568529068506 (gnn_message_passing).
#
# Reference computation (per scene s of 32, P=48 peds):
#   rel[i,j]  = obs[j] - obs[i]                  (P,P,2T)   2T=16
#   emb       = rel @ W_se.T                     (P,P,512)
#   emb      *= tile(traj_weight[s])             (P,P,512)
#   x         = concat([emb, h[j]], -1)          (P,P,576)
#   x1        = relu(x @ W1.T + b1)              (P,P,512)
#   x2        = relu(x1 @ W2.T + b2)             (P,P,1024)
#   out[s,i]  = max_j x2[i,j]                    (P,1024)
#
# Kernel restructuring (validated exactly in fp32 numpy):
#  * The tiled traj_weight multiply + spatial embedding + W1 are fused:
#      out1[d,row] = sum_{(ct,g)} Wf[d,(ct,g)] * tw[row,ct] * rel[row,g]
#    with Wf[d, ct*16+g] = sum_{k%2==c} W1[d, t*64+k] * W_se[t*64+k, g].
#    So MLP1 contracts over 256 "rel2" features (+64 h features) instead
#    of 576, and the (P,P,512) embedding is never materialized.
#  * rel2 = tw_rep * rel_rep is built feature-major on 128 partitions:
#      rel_rep = obs_rep.T @ D   (D = +-1 pairwise difference matrix)
#      tw_rep  = Rb.T @ twT      (Rb = 0/1 replication matrix)
#    i.e. three cheap matmuls + two vector multiplies per row block.
#  * The h-state part of MLP1 rides as a third K-accumulation matmul
#    whose rhs (h broadcast over i) is uploaded once per scene.
#  * relu/bias commute with max-pool, so MLP2 outputs are max-pooled
#    straight out of PSUM; bias+relu are applied post-pool on [128,48].
#  * Matmuls run in bf16 (1 cycle/row; separate LDWEIGHTS path). PSUM
#    accumulation stays fp32; only matmul operands are rounded.
#  * All matmuls use K=128 (operands zero-padded on the host side for
#    the stationary/constant factor; the streamed factor's extra rows
#    are garbage multiplied by zero).  K<128 matmuls with tile_position
#    take a slow SW-decode path on the PE sequencer (~+100ns each).
#
# Sharding: scenes are data-parallel across the 8 cores (4 scenes each);
# weights replicated; the (192,1024) per-core outputs are concatenated on
# the host (no collectives needed).

import numpy as np

S, P, T, E, H = 32, 48, 8, 64, 64
D1, D2 = 512, 1024
B = S * P
NCORES = 8
SC = S // NCORES          # scenes per core
NB = 6                    # row blocks per scene
NBLK = P * P // NB        # 384 columns (pairs) per block = 8 i-groups x 48 j
IB = NBLK // P            # i-groups per block (8)


def _host_constants(W_se, W1, W2, b1, b2):
    """Precompute fused weights + structural constant matrices (fp32)."""
    W_se = np.asarray(W_se, np.float32)
    W1 = np.asarray(W1, np.float32)
    W2 = np.asarray(W2, np.float32)
    b1 = np.asarray(b1, np.float32)
    b2 = np.asarray(b2, np.float32)

    W1e, W1h = W1[:, :512], W1[:, 512:]
    Wf = np.zeros((D1, 256), np.float32)
    for c in range(2):
        for t in range(T):
            ct = c * 8 + t
            f = t * 64 + np.arange(c, 64, 2)
            Wf[:, ct * 16:(ct + 1) * 16] = W1e[:, f] @ W_se[f, :]

    # pairwise difference matrix, zero-padded to K=128 rows
    Dm = np.zeros((128, P * P), np.float32)
    ii, jj = np.meshgrid(np.arange(P), np.arange(P), indexing="ij")
    rows = (ii * P + jj).ravel()
    np.add.at(Dm, (jj.ravel(), rows), 1.0)
    np.add.at(Dm, (ii.ravel(), rows), -1.0)

    # lhsT tile layouts: [128, kTiles, M] so DMAs are contiguous
    Wf_sb = np.ascontiguousarray(Wf.T.reshape(2, 128, D1).transpose(1, 0, 2))
    W1h_sb = np.zeros((128, D1), np.float32)
    W1h_sb[:H] = W1h.T                                       # K=128 zero-pad
    W2_sb = np.ascontiguousarray(W2.T.reshape(4, 128, D2).transpose(1, 0, 2))
    b1_sb = np.ascontiguousarray(b1.reshape(4, 128).T)       # (128, 4)
    b2_sb = np.ascontiguousarray(b2.reshape(8, 128).T)       # (128, 8)
    ident = np.eye(128, dtype=np.float32)
    return dict(Wf_sb=Wf_sb, W1h_sb=W1h_sb, W2_sb=W2_sb, b1_sb=b1_sb,
                b2_sb=b2_sb, Dm=Dm, ident=ident)


def build_program(n_scenes=SC):
    """Emit the per-core Bass/Tile program. Returns the compiled Bacc.

    Built on bacc.Bacc (not raw bass.Bass): Bacc.compile() runs the
    TRN2 sync legalization (move_matmul_waits_to_ldweights +
    generate_event_semaphores) that splits multi-semaphore waits —
    hardware allows at most one sync-wait per instruction.
    """
    from contextlib import ExitStack
    import concourse.bacc as bacc
    import concourse.tile as tile
    from concourse import mybir
    from concourse.alu_op_type import AluOpType

    f32 = mybir.dt.float32
    bf16 = mybir.dt.bfloat16
    AF = mybir.ActivationFunctionType
    AX = mybir.AxisListType

    nc = bacc.Bacc("TRN2", target_bir_lowering=False, debug=False)

    # ---- DRAM parameters -------------------------------------------------
    # bf16 consts blob: Dm(2304) | W1h(512) = 2816 cols
    d_cb = nc.dram_tensor("constsB", [128, 2816], bf16, kind="ExternalInput")
    # f32 consts blob: b1(4) | b2(8) | ident(128) = 140 cols
    d_cf = nc.dram_tensor("constsF", [128, 140], f32, kind="ExternalInput")
    # scene blob: tw_rep0(2304) | tw_rep1(2304) | hj(384) = 4992 cols
    d_blob = nc.dram_tensor("blob", [n_scenes, 128, 4992], bf16, kind="ExternalInput")
    d_obs = nc.dram_tensor("obs_rep", [n_scenes, 128, 128], bf16, kind="ExternalInput")
    # all matmul weights in one blob: Wf(2*512) | W2(4*1024) = 5120 cols
    d_W = nc.dram_tensor("W_all", [128, 5120], bf16, kind="ExternalInput")
    d_out = nc.dram_tensor("out", [n_scenes * P, D2], f32, kind="ExternalOutput")

    with ExitStack() as ctx:
        tc = ctx.enter_context(tile.TileContext(nc))
        consts = ctx.enter_context(tc.tile_pool(name="consts", bufs=1))
        tw_pool = ctx.enter_context(tc.tile_pool(name="tw", bufs=2))
        scene_pool = ctx.enter_context(tc.tile_pool(name="scene", bufs=2))
        blk_pool = ctx.enter_context(tc.tile_pool(name="blk", bufs=3))
        pp = ctx.enter_context(tc.tile_pool(name="pp", bufs=2, space="PSUM"))
        p1 = ctx.enter_context(tc.tile_pool(name="p1", bufs=2, space="PSUM"))
        p2 = ctx.enter_context(tc.tile_pool(name="p2", bufs=4, space="PSUM"))

        cb = consts.tile([128, 2816], bf16)
        cf = consts.tile([128, 140], f32)
        Dm_sb = cb[:, 0:P * P]
        W1h_sb = cb[:, P * P:P * P + D1]
        b1_sb = cf[:, 0:4]
        b2_sb = cf[:, 4:12]
        id_sb = cf[:, 12:140]
        zero_sb = consts.tile([128, P], f32)
        w_all = consts.tile([128, 5120], bf16)
        Wf_sb = w_all[:, 0:2 * D1].rearrange("p (k m) -> p k m", k=2)
        W2_sb = w_all[:, 2 * D1:].rearrange("p (k m) -> p k m", k=4)

        state = {}   # per-scene tiles
        mlp_q = []   # software pipeline: deferred MLP stage

        def scene_setup(s, chunked=False):
            # tiles are [128, ...]: rows past the data are host-zero-padded
            # (hardware SBUF garbage can contain NaN, and 0*NaN = NaN kills
            # the K=128 zero-padding trick otherwise).  obs rides its own
            # small DMA so the first rel matmul isn't gated on the blob.
            obs_rep = scene_pool.tile([128, 128], bf16, tag="obs_rep")
            nc.sync.dma_start(obs_rep[:], d_obs[s])
            blob = tw_pool.tile([128, 4992], bf16, tag="blob")
            if chunked:
                # scene 0 startup: tw_rep0 + tw_rep1-block0 first so the
                # first TTs aren't gated on the full 3.9us blob transfer
                nc.sync.dma_start(blob[:, 0:P * P + NBLK],
                                  d_blob[s, :, 0:P * P + NBLK])
                nc.sync.dma_start(blob[:, P * P + NBLK:],
                                  d_blob[s, :, P * P + NBLK:])
            else:
                nc.sync.dma_start(blob[:], d_blob[s])
            pooled = scene_pool.tile([128, 4, 2 * P], f32, tag="pooled")
            state[s] = dict(blob=blob, obs_rep=obs_rep, pooled=pooled)

        def load_weights():
            # one weight DMA on the (startup-idle) Activation engine ring:
            # the DMA-completion semaphore pool is ~9 deep, so startup must
            # stay under that or later DMAs serialize on sem reuse
            nc.scalar.dma_start(w_all[:], d_W[:])

        def prep(s, b):
            st = state[s]
            c0 = b * NBLK
            rel_ps = pp.tile([128, NBLK], f32, tag="pp")
            nc.tensor.matmul(rel_ps[:], st["obs_rep"][:],
                             Dm_sb[:, c0:c0 + NBLK], start=True, stop=True)
            # rel eviction on the scalar engine (to bf16) frees DVE cycles;
            # tw_rep comes pre-replicated from DRAM, so both rel2 multiplies
            # run on all-SBUF 2-byte operands (fast DVE mode)
            rel_sb = blk_pool.tile([128, NBLK], bf16, tag="rel_sb")
            nc.scalar.copy(rel_sb[:], rel_ps[:])
            rel2_0 = blk_pool.tile([128, NBLK], bf16, tag="rel2_0")
            nc.vector.tensor_tensor(rel2_0[:], st["blob"][:, c0:c0 + NBLK],
                                    rel_sb[:], AluOpType.mult)
            rel2_1 = blk_pool.tile([128, NBLK], bf16, tag="rel2_1")
            nc.vector.tensor_tensor(rel2_1[:], st["blob"][:, P * P + c0:P * P + c0 + NBLK],
                                    rel_sb[:], AluOpType.mult)
            return dict(rel2_0=rel2_0, rel2_1=rel2_1, s=s, b=b)

        def mlp1(job):
            s, b = job["s"], job["b"]
            st = state[s]
            r20 = job["rel2_0"][:]
            r21 = job["rel2_1"][:]
            x1 = blk_pool.tile([128, 4, NBLK], bf16, tag="x1")
            for m in range(4):
                p1t = p1.tile([128, NBLK], f32, tag="p1")
                nc.tensor.matmul(p1t[:], Wf_sb[:, 0, m * 128:(m + 1) * 128],
                                 r20, start=True, stop=False)
                nc.tensor.matmul(p1t[:], Wf_sb[:, 1, m * 128:(m + 1) * 128],
                                 r21, start=False, stop=False)
                nc.tensor.matmul(p1t[:], W1h_sb[:, m * 128:(m + 1) * 128],
                                 st["blob"][:, 2 * P * P:2 * P * P + NBLK],
                                 start=False, stop=True)
                nc.scalar.activation(x1[:, m, :], p1t[:], AF.Relu,
                                     bias=b1_sb[:, m:m + 1])
            job["x1"] = x1

        def mlp2(job):
            s, b = job["s"], job["b"]
            st = state[s]
            x1 = job["x1"]
            last = b == NB - 1
            for mm in range(8):
                p2t = p2.tile([128, NBLK], f32, tag="p2")
                for k in range(4):
                    nc.tensor.matmul(
                        p2t[:], W2_sb[:, k, mm * 128:(mm + 1) * 128],
                        x1[:, k, :], start=(k == 0), stop=(k == 3))
                dst = st["pooled"][:, mm // 2,
                                   (mm % 2) * P + b * IB:(mm % 2) * P + (b + 1) * IB]
                nc.vector.tensor_reduce(
                    dst, p2t[:].rearrange("p (i j) -> p i j", i=IB),
                    axis=AX.X, op=AluOpType.max)
                # one-group delay so PE doesn't stall on the pair's
                # reduce -> transpose chain (no delay on the final scene,
                # where it would only stretch the kernel tail)
                if last and mm % 2 == 1:
                    if s == n_scenes - 1:
                        finish_pair(s, st, mm // 2)
                    elif mm >= 3:
                        finish_pair(s, st, (mm - 3) // 2)
            if last:
                if s != n_scenes - 1:
                    finish_pair(s, st, 2)
                    finish_pair(s, st, 3)
                state.pop(s)

        def finish_pair(s, st, pi):
            """Scene output for m-tile pair pi: bias+relu post-pool,
            transpose to row-major, DMA straight out of PSUM."""
            pooled = st["pooled"]
            for half in range(2):
                mm = 2 * pi + half
                sl = pooled[:, pi, half * P:(half + 1) * P]
                nc.vector.scalar_tensor_tensor(
                    sl, sl, b2_sb[:, mm:mm + 1], zero_sb[:],
                    op0=AluOpType.add, op1=AluOpType.max)
            tps = p1.tile([128, NBLK], f32, tag="p1")
            nc.tensor.transpose(tps[:2 * P, :128], pooled[:, pi, :], id_sb)
            ot = scene_pool.tile([2 * P, 128], f32, tag="ot")
            nc.vector.tensor_copy(ot[:], tps[:2 * P, :128])
            nc.sync.dma_start(
                d_out[s * P:(s + 1) * P, (2 * pi) * 128:(2 * pi + 1) * 128],
                ot[:P, :])
            nc.sync.dma_start(
                d_out[s * P:(s + 1) * P, (2 * pi + 1) * 128:(2 * pi + 2) * 128],
                ot[P:2 * P, :])

        # startup order: the consts blob holds Dm/Rb needed by the very
        # first prep matmul, so it dispatches first on the sync queue;
        # scene-0 data follows; weights ride the (startup-idle) scalar
        # HWDGE ring in parallel; the STT zero operand memset is on vector
        nc.sync.dma_start(cb[:], d_cb[:])
        scene_setup(0, chunked=True)
        nc.sync.dma_start(cf[:], d_cf[:])
        nc.vector.memset(zero_sb[:], 0.0)
        load_weights()

        # two-deep software pipeline on PE:
        #   ... prep(i)  mlp1(i-1)  mlp2(i-2) ...
        # so x1 is ready a full block before MLP2 consumes it and PSUM
        # slot recycling has a block of slack; scene data is prefetched
        # one block before the scene starts
        blocks = [(s, b) for s in range(n_scenes) for b in range(NB)]
        for idx, (s, b) in enumerate(blocks):
            if b == NB - 2 and s + 1 < n_scenes:
                scene_setup(s + 1)
            mlp_q.append(prep(s, b))
            if len(mlp_q) > 1:
                mlp1(mlp_q[-2])
            if len(mlp_q) > 2:
                mlp2(mlp_q.pop(0))
        mlp1(mlp_q[-1])
        mlp2(mlp_q.pop(0))
        mlp2(mlp_q.pop(0))

    nc.compile()
    return nc


def _host_inputs(h_states, traj, traj_weight, consts, n_scenes=SC):
    """Slice + lay out per-core input maps (matmul operands cast to bf16)."""
    import ml_dtypes
    bf = ml_dtypes.bfloat16
    h_states = np.asarray(h_states, np.float32)
    traj = np.asarray(traj, np.float32)
    traj_weight = np.asarray(traj_weight, np.float32)

    obs_full = np.ascontiguousarray(
        traj[:T].transpose(1, 0, 2).reshape(B, 2 * T))          # (B,16) g=t*2+c
    h_full = h_states.reshape(S, P, H)

    constsB = np.concatenate(
        [consts["Dm"], consts["W1h_sb"]], axis=1).astype(bf)    # (128, 2816)
    constsF = np.concatenate(
        [consts["b1_sb"], consts["b2_sb"], consts["ident"]],
        axis=1).astype(np.float32)                              # (128, 140)
    Wf_sb = consts["Wf_sb"].astype(bf)
    W2_sb = consts["W2_sb"].astype(bf)

    in_maps = []
    for core in range(NCORES):
        s0 = core * n_scenes
        sl = slice(s0, s0 + n_scenes)
        twT = traj_weight[sl].transpose(0, 2, 3, 1).reshape(n_scenes, 16, P * P)
        # blob: tw_rep0 | tw_rep1 | hj.  tw_rep{t}[p] = tw[8t + p//16]
        blob = np.zeros((n_scenes, 128, 4992), np.float32)
        blob[:, :, 0:P * P] = np.repeat(twT[:, 0:8], 16, axis=1)
        blob[:, :, P * P:2 * P * P] = np.repeat(twT[:, 8:16], 16, axis=1)
        # hj[s, d, r*48+j] = h[s, j, d] broadcast over i-groups (rows 64+ zero)
        blob[:, :H, 2 * P * P:] = np.tile(
            h_full[sl].transpose(0, 2, 1).reshape(n_scenes, H, 1, P),
            (1, 1, IB, 1)).reshape(n_scenes, H, NBLK)
        blob = blob.astype(bf)
        # obs_rep[s, j, r*16+g] = obs[j, g] replicated 8x along features
        obs_rep = np.zeros((n_scenes, 128, 128), np.float32)
        obs_rep[:, :P] = np.tile(
            obs_full[s0 * P:(s0 + n_scenes) * P].reshape(n_scenes, P, 1, 2 * T),
            (1, 1, 8, 1)).reshape(n_scenes, P, 128)
        obs_rep = obs_rep.astype(bf)
        m = dict(blob=blob, obs_rep=obs_rep, constsB=constsB, constsF=constsF,
                 W_all=np.concatenate([Wf_sb.reshape(128, 2 * D1),
                                       W2_sb.reshape(128, 4 * D2)], axis=1))
        in_maps.append(m)
    return in_maps


def kernel(h_states, seq_start_end, end_pos, traj, traj_weight,
           mlp_pre_pool_dim_0, W_se, b_se, W1, b1, W2, b2):
    import sys
    if '/opt/trn_rl_repo' not in sys.path:
        sys.path.insert(0, '/opt/trn_rl_repo')
    from concourse.bass_utils import run_bass_kernel_spmd

    consts = _host_constants(W_se, W1, W2, b1, b2)
    in_maps = _host_inputs(h_states, traj, traj_weight, consts)
    nc = build_program(SC)
    res = run_bass_kernel_spmd(nc, in_maps, list(range(NCORES)))
    out = np.concatenate([res.results[i]["out"] for i in range(NCORES)], axis=0)
    return out.astype(np.float32)
